# revision 14
# baseline (speedup 1.0000x reference)
"""Fused multi-head attention block on 8 TRN2 NeuronCores.

reference: qkv = x@Wqkv+b; q,k rmsnorm'd per head; softmax(q k^T/sqrt(hd)) v; proj.
Shapes: x [2,2048,1024], H=16 heads, hd=64.

Distribution (no collectives): 8 cores = 2 batches x 4 head-groups (4 heads each).
Core c: batch b=c//4, heads 4g..4g+3 (g=c%4). Each core computes the partial
projection output (proj_w row-sharded over its heads) for its batch; the host
sums the 4 partials per batch and adds proj_b.

Per-core pipeline (all matmuls float32r at 1 cyc/row, AV in bf16):
  A) x tile -> PE-transpose -> xT ; qkv GEMM (qk N=512, v N=256) ; rmsnorm(q,k)
     with rstd = exp(-0.5 ln(mean+eps)) (single ACT table set);
     PE-transpose normed qk -> qT,kT (norm weights folded into the evict);
     v (+bias) -> bf16 v_aug with a ones column per head (softmax denominator).
  B) per (head, qtok-half, ktile): S^T = kT^T qT (2 MMs into a 2-bank psum),
     one exp (scale=1/8) -> P^T bf16, 8 AV MMs accumulate [qtok,64+1] in psum.
     No max-subtraction (scores in [-6,6] -> exp safe in fp32/bf16).
     Epilogue: attnout = AV / denom (per-partition reciprocal mul).
  C) PE-transpose attnout -> aT; partial proj GEMM; DMA out [2048,1024].
"""

from contextlib import ExitStack

import numpy as np

import concourse.bass as bass
import concourse.mybir as mybir
import concourse.tile as tile
from concourse import bacc
from concourse.bass_utils import run_bass_kernel_spmd
from concourse.masks import make_identity

B, N, C = 2, 2048, 1024
H, HD = 16, 64
HPC = 4                 # heads per core
NT = N // 128           # 16 token tiles
KT8 = C // 128          # 8 contraction tiles for the qkv GEMM
QK = 2 * HPC * HD       # 512 qk channels per core
V = HPC * HD            # 256 v channels per core
EPS = 1e-6
F32 = mybir.dt.float32
F32R = mybir.dt.float32r
BF16 = mybir.dt.bfloat16
AF = mybir.ActivationFunctionType


def _r(ap):
    return ap.bitcast(F32R)


def build_nc(dbg=False):
    nc = bacc.Bacc("TRN2", target_bir_lowering=False, debug=False)

    x_ext = nc.declare_dram_parameter("x", [N, C], F32, isOutput=False)
    wqkv_ext = nc.declare_dram_parameter("wqkv", [C, QK + V], F32R, isOutput=False)
    bqkv_ext = nc.declare_dram_parameter("bqkv", [QK + V], F32, isOutput=False)
    normw_ext = nc.declare_dram_parameter("normw", [QK], F32, isOutput=False)
    wproj_ext = nc.declare_dram_parameter("wproj", [V, C], F32R, isOutput=False)
    out_ext = nc.declare_dram_parameter("out", [N, C], F32, isOutput=True)
    if dbg:
        dbg_q = nc.declare_dram_parameter("dbg_q", [128, 2, N], F32, isOutput=True)
        dbg_k = nc.declare_dram_parameter("dbg_k", [128, 2, N], F32, isOutput=True)
        dbg_v = nc.declare_dram_parameter("dbg_v", [128, NT, HPC, HD + 1], F32, isOutput=True)
        dbg_a = nc.declare_dram_parameter("dbg_a", [128, NT, HPC, HD], F32, isOutput=True)

    with tile.TileContext(nc) as tc, ExitStack() as ctx:
        singles = ctx.enter_context(tc.tile_pool(name="singles", bufs=1))

        ident = singles.tile([128, 128], F32, tag="ident")
        make_identity(nc, ident)
        eps_sb = singles.tile([128, 1], F32, tag="eps")
        nc.vector.memset(eps_sb, EPS)

        wqkv_sb = singles.tile([128, KT8, QK + V], F32R, tag="wqkv")
        for kt in range(KT8):
            nc.sync.dma_start(
                out=wqkv_sb[:, kt, :], in_=wqkv_ext[kt * 128:(kt + 1) * 128, :]
            )
        wproj_sb = singles.tile([128, 2, C], F32R, tag="wproj")
        for rb in range(2):
            nc.sync.dma_start(
                out=wproj_sb[:, rb, :], in_=wproj_ext[rb * 128:(rb + 1) * 128, :]
            )
        bias_sb = singles.tile([128, QK + V], F32, tag="bias")
        nc.sync.dma_start(out=bias_sb, in_=bqkv_ext[:].partition_broadcast(128))
        # normw_sb[p, cb] = normw[cb*128 + p]
        normw_sb = singles.tile([128, 4], F32, tag="normw")
        nc.sync.dma_start(out=normw_sb, in_=normw_ext[:].rearrange("(b p) -> p b", p=128))

        # persistent activations
        qT = singles.tile([128, 2, N], F32R, tag="qT")     # channel-major q
        kT = singles.tile([128, 2, N], F32R, tag="kT")     # channel-major k
        vaug = singles.tile([128, NT, HPC, HD + 1], BF16, tag="vaug")
        attnout = singles.tile([128, NT, HPC, HD], F32, tag="attnout")
        aT = singles.tile([128, 2, N], F32R, tag="aT")

        # ones column for the softmax denominator (cols 0..63 overwritten per tile)
        nc.vector.memset(vaug, 1.0)

        # ---------------- phase A: qkv + rmsnorm + transposes ----------------
        with ExitStack() as actx:
            xpool = actx.enter_context(tc.tile_pool(name="xin", bufs=3))
            xtpool = actx.enter_context(tc.tile_pool(name="xt", bufs=2))
            qkpool = actx.enter_context(tc.tile_pool(name="qksb", bufs=3))
            stpool = actx.enter_context(tc.tile_pool(name="stats", bufs=6))
            ptr = actx.enter_context(tc.tile_pool(name="ptr", bufs=2, space="PSUM"))
            pqk = actx.enter_context(tc.tile_pool(name="pqk", bufs=1, space="PSUM"))
            pv = actx.enter_context(tc.tile_pool(name="pv", bufs=1, space="PSUM"))

            for t in range(NT):
                ts = slice(t * 128, (t + 1) * 128)
                xin = xpool.tile([128, C], F32, tag="xin")
                nc.sync.dma_start(out=xin, in_=x_ext[ts, :])

                # transpose x tile -> xT blocks [C-part, tok]
                xt = xtpool.tile([128, KT8, 128], F32R, tag="xt")
                for kt in range(KT8):
                    p_tr = ptr.tile([128, 128], F32, tag="ptr")
                    nc.tensor.transpose(p_tr, xin[:, kt * 128:(kt + 1) * 128], ident)
                    if kt % 2 == 0:
                        nc.vector.tensor_copy(xt[:, kt, :], p_tr)
                    else:
                        nc.scalar.copy(xt[:, kt, :], p_tr)

                # qkv GEMM for this token tile
                p_qk = pqk.tile([128, QK], F32, tag="pqk")
                p_v = pv.tile([128, V], F32, tag="pv")
                for kt in range(KT8):
                    nc.tensor.matmul(
                        p_qk, xt[:, kt, :], wqkv_sb[:, kt, 0:QK],
                        start=(kt == 0), stop=(kt == KT8 - 1),
                    )
                for kt in range(KT8):
                    nc.tensor.matmul(
                        p_v, xt[:, kt, :], wqkv_sb[:, kt, QK:QK + V],
                        start=(kt == 0), stop=(kt == KT8 - 1),
                    )

                # eviction with bias add
                qk_sb = qkpool.tile([128, QK], F32, tag="qksb")
                nc.vector.tensor_add(qk_sb, p_qk, bias_sb[:, 0:QK])
                nc.vector.tensor_add(
                    vaug[:, t, :, 0:HD],
                    p_v.rearrange("p (h d) -> p h d", d=HD),
                    bias_sb[:, QK:QK + V].rearrange("p (h d) -> p h d", d=HD),
                )

                # rmsnorm over each 64-channel head group of q and k
                sq = qkpool.tile([128, QK], F32, tag="sq")
                nc.vector.tensor_mul(sq, qk_sb, qk_sb)
                ssq = stpool.tile([128, 2 * HPC], F32, tag="ssq")
                nc.vector.tensor_reduce(
                    ssq, sq.rearrange("p (g d) -> p g d", d=HD),
                    axis=mybir.AxisListType.X, op=mybir.AluOpType.add,
                )
                lnv = stpool.tile([128, 2 * HPC], F32, tag="lnv")
                nc.scalar.activation(lnv, ssq, AF.Ln, bias=eps_sb, scale=1.0 / HD)
                rstd = stpool.tile([128, 2 * HPC], F32, tag="rstd")
                nc.scalar.activation(rstd, lnv, AF.Exp, scale=-0.5)
                for g in range(2 * HPC):
                    nc.vector.tensor_scalar_mul(
                        qk_sb[:, g * HD:(g + 1) * HD],
                        qk_sb[:, g * HD:(g + 1) * HD],
                        rstd[:, g:g + 1],
                    )

                # transpose normed qk -> qT/kT, folding in the norm weights
                for cb in range(4):
                    p_tr = ptr.tile([128, 128], F32, tag="ptr")
                    nc.tensor.transpose(p_tr, qk_sb[:, cb * 128:(cb + 1) * 128], ident)
                    dst = qT[:, cb, ts] if cb < 2 else kT[:, cb - 2, ts]
                    nc.vector.tensor_scalar_mul(dst, p_tr, normw_sb[:, cb:cb + 1])

        # ---------------- phase B: attention ----------------
        with ExitStack() as bctx:
            spool = bctx.enter_context(tc.tile_pool(name="ps", bufs=2, space="PSUM"))
            opool = bctx.enter_context(tc.tile_pool(name="po", bufs=2, space="PSUM"))
            ptpool = bctx.enter_context(tc.tile_pool(name="pt", bufs=3))
            rpool = bctx.enter_context(tc.tile_pool(name="rec", bufs=8))

            for h in range(HPC):
                pb = (h % 2) * 64          # partition base of this head's channels
                cb = h // 2                # column block in qT/kT
                psl = slice(pb, pb + 64)
                for qh in range(2):        # 1024-token halves of the query axis
                    po = opool.tile([128, 8, 128], F32, tag="po")
                    for kt in range(NT):
                        ps = spool.tile([128, 1024], F32, tag="ps")
                        for i in range(2):
                            qsl = slice(qh * 1024 + i * 512, qh * 1024 + (i + 1) * 512)
                            nc.tensor.matmul(
                                ps[:, i * 512:(i + 1) * 512],
                                kT[psl, cb, kt * 128:(kt + 1) * 128],
                                qT[psl, cb, qsl],
                                start=True, stop=True,
                            )
                        pt = ptpool.tile([128, 1024], BF16, tag="pt")
                        nc.scalar.activation(pt, ps, AF.Exp, scale=0.125)
                        for qs in range(8):
                            # start=True clears has_written for the whole PSUM
                            # bank, so only the first accumulator touching each
                            # bank (4 share a bank) may set it; the others
                            # overwrite-on-first-touch via cleared has_written.
                            nc.tensor.matmul(
                                po[:, qs, 0:HD + 1],
                                pt[:, qs * 128:(qs + 1) * 128],
                                vaug[:, kt, h, :],
                                start=(kt == 0 and qs % 4 == 0),
                                stop=(kt == NT - 1 and qs % 4 == 3),
                            )
                    for qs in range(8):
                        rec = rpool.tile([128, 1], F32, tag="rec")
                        nc.vector.reciprocal(rec, po[:, qs, HD:HD + 1])
                        nc.vector.tensor_scalar_mul(
                            attnout[:, qh * 8 + qs, h, :], po[:, qs, 0:HD], rec
                        )

        # ---------------- phase C: transpose + partial projection ----------------
        with ExitStack() as cctx:
            ptr2 = cctx.enter_context(tc.tile_pool(name="ptr2", bufs=2, space="PSUM"))
            ppool = cctx.enter_context(tc.tile_pool(name="pp", bufs=2, space="PSUM"))
            outpool = cctx.enter_context(tc.tile_pool(name="outsb", bufs=4))

            for t in range(NT):
                ts = slice(t * 128, (t + 1) * 128)
                for rb in range(2):
                    p_tr = ptr2.tile([128, 128], F32, tag="ptr2")
                    nc.tensor.transpose(
                        p_tr,
                        attnout[:, t, 2 * rb:2 * rb + 2, :].rearrange("p h d -> p (h d)"),
                        ident,
                    )
                    nc.scalar.copy(aT[:, rb, ts], p_tr)
            for t in range(NT):
                ts = slice(t * 128, (t + 1) * 128)
                for jg in range(2):
                    pp = ppool.tile([128, 512], F32, tag="pp")
                    for rb in range(2):
                        nc.tensor.matmul(
                            pp, aT[:, rb, ts],
                            wproj_sb[:, rb, jg * 512:(jg + 1) * 512],
                            start=(rb == 0), stop=(rb == 1),
                        )
                    outsb = outpool.tile([128, 512], F32, tag="outsb")
                    nc.vector.tensor_copy(outsb, pp)
                    nc.sync.dma_start(
                        out=out_ext[ts, jg * 512:(jg + 1) * 512], in_=outsb
                    )

        if dbg:
            nc.sync.dma_start(out=dbg_q[:, :, :], in_=qT.bitcast(F32))
            nc.sync.dma_start(out=dbg_k[:, :, :], in_=kT.bitcast(F32))
            with tc.tile_pool(name="dbgv", bufs=1) as dv:
                vf = dv.tile([128, NT, HPC, HD + 1], F32, tag="vf")
                nc.vector.tensor_copy(vf, vaug)
                nc.sync.dma_start(out=dbg_v[:, :, :, :], in_=vf)
            nc.sync.dma_start(out=dbg_a[:, :, :, :], in_=attnout)

    nc.finalize()
    return nc


def make_in_maps(x, qkv_w, qkv_b, q_norm_w, k_norm_w, proj_w, proj_b):
    """Shard the full inputs into the 8 per-core input maps."""
    in_maps = []
    for c in range(8):
        b, g = c // 4, c % 4
        ch = np.arange(4 * g * HD, 4 * (g + 1) * HD)          # this core's head channels
        wqkv_c = np.concatenate(
            [qkv_w[:, ch], qkv_w[:, C + ch], qkv_w[:, 2 * C + ch]], axis=1
        )
        bqkv_c = np.concatenate([qkv_b[ch], qkv_b[C + ch], qkv_b[2 * C + ch]])
        normw = np.concatenate([np.tile(q_norm_w, HPC), np.tile(k_norm_w, HPC)])
        in_maps.append({
            "x": np.ascontiguousarray(x[b], np.float32),
            "wqkv": np.ascontiguousarray(wqkv_c, np.float32),
            "bqkv": np.ascontiguousarray(bqkv_c, np.float32),
            "normw": np.ascontiguousarray(normw, np.float32),
            "wproj": np.ascontiguousarray(proj_w[ch, :], np.float32),
        })
    return in_maps


_NC_CACHE = []


def kernel(x, qkv_w, qkv_b, q_norm_w, k_norm_w, proj_w, proj_b,
           _run_kwargs=None, _res_box=None):
    x = np.asarray(x); qkv_w = np.asarray(qkv_w); qkv_b = np.asarray(qkv_b)
    q_norm_w = np.asarray(q_norm_w); k_norm_w = np.asarray(k_norm_w)
    proj_w = np.asarray(proj_w); proj_b = np.asarray(proj_b)

    if not _NC_CACHE:
        _NC_CACHE.append(build_nc())
    nc = _NC_CACHE[0]
    in_maps = make_in_maps(x, qkv_w, qkv_b, q_norm_w, k_norm_w, proj_w, proj_b)
    res = run_bass_kernel_spmd(nc, in_maps, core_ids=list(range(8)),
                               **(_run_kwargs or {}))
    if _res_box is not None:
        _res_box["res"] = res
    out = np.zeros((B, N, C), np.float32)
    for c in range(8):
        out[c // 4] += res.results[c]["out"]
    out += proj_b[None, None, :].astype(np.float32)
    return out


if __name__ == "__main__":
    rng = np.random.default_rng(0)
    x = rng.standard_normal((B, N, C), np.float32)
    qkv_w = (rng.standard_normal((C, 3 * C), np.float32) / np.sqrt(C)).astype(np.float32)
    qkv_b = np.zeros((3 * C,), np.float32)
    qn = np.ones((HD,), np.float32)
    kn = np.ones((HD,), np.float32)
    proj_w = (rng.standard_normal((C, C), np.float32) / np.sqrt(C)).astype(np.float32)
    proj_b = np.zeros((C,), np.float32)
    out = kernel(x, qkv_w, qkv_b, qn, kn, proj_w, proj_b)
    print("out", out.shape, out.dtype, float(np.abs(out).mean()))
